# revision 1
# baseline (speedup 1.0000x reference)
"""Trainium2 Bass kernel for nn_DCGN_78967268704510.

Math: the reference's get_adjacent() builds a diagonal matrix (the faithful
buggy triple loop zeroes every off-diagonal), adds I, then symmetric-
normalizes; for a diagonal matrix D^-1/2 A D^-1/2 == I exactly (to fp32
ulps).  attn_pool feeds only get_adjacent, so the whole network collapses
to two fused stages:

  h   = leaky( (sum_p x[:,4s+p,:] * conv1_w[p,:]) @ prop1_W + prop1_B )
  out = leaky( (sum_p h[:,4t+p,:] * conv2_w[p,:]) @ prop2_W + prop2_B )

Verified vs the full jax reference: absmax err 8e-08 (2.7e-6 of scale).

Sharding: pure data parallel, batch 64 -> 8 cores x 8 batches each.

Per-core pipeline, software-pipelined so stage_b(b-1) PE work interleaves
with stage_a(b) and the PE never waits on the DVE scale-muls:
  - DMA x[b] as 4 row tiles c_q [128n, 2048f] (batches 0/1 prefetched
    ahead of the big weight streams)
  - DVE+GpSimd: c_q *= replicated conv1_w pattern (rounds to float32r)
  - PE pool (fp32r, 1cyc/row): accumulate G_q.T @ c_q over q with
    per-q selector matrices G_q[n,s]=1 iff s==32q+n//4 -> xc [128s, f]
  - PE transpose-mode vs identity (fp32r, 1.5cyc/row) -> xcT [f, s]
  - mm1 (fp32r): 16 K-tile matmuls vs prop1_W, N chunks 384/384/332
  - +bias (DVE), LeakyRelu alpha=.01 (ACT), *conv2_w pattern (DVE)
  - stage-2 pool-transpose matmuls vs G, 4-batch-packed mm2 vs prop2_W,
    +bias, leaky, DMA out.

float32r matmuls measure ~1.4e-4 relative-to-scale rounding on K=2048
(vs 8e-5 for pure-fp32 numpy); end-to-end output absmax error is
2.8e-05 of output scale.  All non-matmul arithmetic is fp32.

HW exec time: ~194 us/core (8 cores run the same SPMD program).
"""
import sys

if '/opt/trn_rl_repo' not in sys.path:
    sys.path.insert(0, '/opt/trn_rl_repo')

import numpy as np

import concourse.bass as bass
import concourse.mybir as mybir
import concourse.tile as tile
from concourse.bass_utils import run_bass_kernel_spmd
from concourse.vector_clock import ScopedClock

N_CORES = 8
B, N, F, HID, NCLASS, P = 64, 512, 2048, 1100, 512, 4
BPC = B // N_CORES          # 8 batches per core
S = N // P                  # 128 stage-1 nodes
T = S // P                  # 32 stage-2 nodes
FT = F // 128               # 16 f-tiles
JT = (HID + 127) // 128     # 9 j-tiles, last has 76 rows
JLAST = HID - 128 * (JT - 1)
MM1_CHUNKS = (384, 384, 332)   # all >=256 so float32r runs at 1 cyc/row

FP32 = mybir.dt.float32
F32R = mybir.dt.float32r


class PatchedTileContext(tile.TileContext):
    """This container's walrus refuses ANY instruction carrying >1 sync
    wait (the TPB EVENTS struct has a single wait slot and the codegen
    won't split).  Split every multi-wait instruction into single-wait
    same-engine nops followed by the instruction with its last wait."""

    def _split_waits(self, inst):
        si = inst.sync_info
        waits = list(si.on_wait) if si and si.on_wait else []
        if len(waits) <= 1:
            return
        for w in waits[:-1]:
            nop = mybir.InstNoOp(
                name=self.nc.get_next_instruction_name(), ins=[], outs=[]
            )
            nop.engine = inst.engine
            nop.sync_info = mybir.SyncInfo(on_wait=[w], on_update=[])
            nop.bass_nofuse = True
            self._add_instruction(nop)
        inst.sync_info = mybir.SyncInfo(
            on_wait=[waits[-1]], on_update=list(si.on_update or [])
        )

    def _commit_instruction(self, inst, lazy_reg_writes=True):
        if inst.engine != mybir.EngineType.Unassigned:
            self._split_waits(inst)
        return super()._commit_instruction(inst, lazy_reg_writes)

    def _drain_and_barrier(self, tick_clock, wait_clock):
        probe = self.nc.sync.nop()
        wait_clock.add_sem_waits(
            probe.ins, ScopedClock({None: tick_clock.global_clock})
        )
        si = probe.ins.sync_info
        waits = list(si.on_wait) if si and si.on_wait else []
        if si and waits:
            probe.ins.sync_info = mybir.SyncInfo(
                on_wait=waits[:1], on_update=list(si.on_update or [])
            )
        for w in waits[1:]:
            n2 = self.nc.sync.nop()
            n2.ins.sync_info = mybir.SyncInfo(on_wait=[w], on_update=[])
        self.nc.sync.drain()
        self.nc.all_engine_barrier()
        assert self.sems is not None
        popped = self.nc._tile_sem_poison_stack.pop()
        assert popped is self._sem_poison
        self.nc.clear_and_free_semaphores(list(self.sems.allocated().values()))
        self.nc.all_engine_barrier()


def build_nc():
    nc = bass.Bass()
    xs_d = nc.dram_tensor('xs', [BPC, N, F], F32R, kind='ExternalInput')
    w1rep_d = nc.dram_tensor('w1rep', [128, F], FP32, kind='ExternalInput')
    gpad_d = nc.dram_tensor('gpad', [128, 224], F32R, kind='ExternalInput')
    id_d = nc.dram_tensor('ident', [128, 128], F32R, kind='ExternalInput')
    w1p_d = nc.dram_tensor('w1p', [F, HID], F32R, kind='ExternalInput')
    b1rep_d = nc.dram_tensor('b1rep', [128, HID], FP32, kind='ExternalInput')
    w2rep_d = nc.dram_tensor('w2rep', [128, HID], FP32, kind='ExternalInput')
    w2p_d = nc.dram_tensor('w2p', [HID, NCLASS], F32R, kind='ExternalInput')
    b2rep_d = nc.dram_tensor('b2rep', [128, NCLASS], FP32, kind='ExternalInput')
    y_d = nc.dram_tensor('y', [BPC, T, NCLASS], FP32, kind='ExternalOutput')
    y_flat = y_d.rearrange('b t c -> (b t) c')   # [256, 512]

    with PatchedTileContext(nc) as tc:
        with (
            tc.tile_pool(name='wpool', bufs=1) as wpool,
            tc.tile_pool(name='cpool', bufs=7) as cpool,
            tc.tile_pool(name='xcpool', bufs=2) as xcpool,
            tc.tile_pool(name='xcTpool', bufs=4) as xcTpool,
            tc.tile_pool(name='h2pool', bufs=2) as h2pool,
            tc.tile_pool(name='hcTpool', bufs=2) as hcTpool,
            tc.tile_pool(name='opool', bufs=1) as opool,
            tc.tile_pool(name='pbig', bufs=2, space='PSUM') as pbigpool,
            tc.tile_pool(name='ph', bufs=3, space='PSUM') as phpool,
            tc.tile_pool(name='p2', bufs=1, space='PSUM') as p2pool,
        ):
            # ---- PE warmup: dummy matmuls on scratch SBUF during the
            #      initial DMA wait so HAM reaches K=8/8 before batch 0 ----

            # ---- DMA order: constants for batch 0, then x0/x1/x2, the
            #      W1 stream interleaved with x3 so the tensor engine has
            #      pool/transpose work while W1 is still in flight, then
            #      the stage-2 weights, then x4..x7 (prefetched in-loop).
            w1rep = wpool.tile([128, F], FP32, tag='w1rep')
            nc.sync.dma_start(out=w1rep[:], in_=w1rep_d[:])
            # gpad[m, 96 + m//4] = 1: column windows gpad[:, 96-32q:224-32q]
            # reproduce the per-chunk pool selector blocks; gpad[:, 96:128]
            # is the stage-2 selector.
            gpad = wpool.tile([128, 224], F32R, tag='gpad')
            nc.sync.dma_start(out=gpad[:], in_=gpad_d[:])
            ident = wpool.tile([128, 128], F32R, tag='ident')
            nc.sync.dma_start(out=ident[:], in_=id_d[:])
            warm_ps = pbigpool.tile([128, 512], FP32, tag='pbig',
                                    name='warm_ps')
            for i in range(32):
                nc.tensor.matmul(
                    warm_ps[:, 0:128], ident[:], ident[:],
                    start=(i == 0), stop=(i == 31),
                )

            def load_c(b):
                cq = []
                for q in range(4):
                    c = cpool.tile([128, F], F32R, tag='c',
                                   name=f'c_{b}_{q}')
                    nc.sync.dma_start(
                        out=c[:], in_=xs_d[b, q * 128:(q + 1) * 128, :]
                    )
                    cq.append(c)
                return cq

            cq_pre = {0: load_c(0), 1: load_c(1)}

            w1ps = []
            for k in range(FT):
                wt = wpool.tile([128, HID], F32R, tag=f'w1p{k}')
                nc.sync.dma_start(
                    out=wt[:], in_=w1p_d[k * 128:(k + 1) * 128, :]
                )
                w1ps.append(wt)
            b1rep = wpool.tile([128, HID], FP32, tag='b1rep')
            nc.sync.dma_start(out=b1rep[:], in_=b1rep_d[:])
            w2rep = wpool.tile([128, HID], FP32, tag='w2rep')
            nc.sync.dma_start(out=w2rep[:], in_=w2rep_d[:])

            # b2rep/w2p are not needed until mm2 of group 0 (iteration 4);
            # defer their DMAs behind x2..x4 so the x stream starts sooner.
            b2rep = wpool.tile([128, NCLASS], FP32, tag='b2rep')
            w2p = wpool.tile([128, JT * NCLASS], F32R, tag='w2p')

            def load_w2():
                nc.sync.dma_start(out=b2rep[:], in_=b2rep_d[:])
                for m in range(JT):
                    rows = 128 if m < JT - 1 else JLAST
                    nc.sync.dma_start(
                        out=w2p[0:rows, m * NCLASS:(m + 1) * NCLASS],
                        in_=w2p_d[m * 128:m * 128 + rows, :],
                    )

            hcT = [None, None]
            xcTs = {}

            xcs = {}

            def scale_c(cq):
                """conv1 scale split 4/4 across DVE and gpsimd: under heavy
                DMA both engines degrade to ~3us per [128,1024] half (SBUF
                bandwidth contention), so an even split minimizes the
                critical path."""
                for i, (q, h_) in enumerate(
                    (q, h_) for q in range(4) for h_ in range(2)
                ):
                    eng = nc.vector if i % 2 == 0 else nc.gpsimd
                    sl = slice(1024 * h_, 1024 * (h_ + 1))
                    eng.tensor_mul(cq[q][:, sl], cq[q][:, sl],
                                   w1rep[:, sl])

            def stage_a1(b, cq):
                """pool -> xc(b) [s, f] (cq already scaled)."""
                xc = [xcpool.tile([128, 1024], F32R, tag='xc',
                                  name=f'xc_{b}_{i}') for i in range(2)]
                for hf in range(2):
                    pp = pbigpool.tile([128, 1024], FP32, tag='pbig',
                                       name=f'pp_{b}_{hf}')
                    for c2 in range(2):
                        ch = 2 * hf + c2
                        for q in range(4):
                            nc.tensor.matmul(
                                pp[:, 512 * c2:512 * (c2 + 1)],
                                gpad[:, 96 - 32 * q:224 - 32 * q],
                                cq[q][:, 512 * ch:512 * (ch + 1)],
                                start=(q == 0), stop=(q == 3),
                            )
                    nc.scalar.copy(out=xc[hf][:], in_=pp[:])
                xcs[b] = xc

            def stage_a2(b):
                """transpose xc(b) -> xcT(b) [f, s]."""
                xc = xcs.pop(b)
                xcT = [xcTpool.tile([128, 1024], F32R, tag='xcT',
                                    name=f'xcT_{b}_{i}') for i in range(2)]
                for hf in range(2):
                    pt = pbigpool.tile([128, 1024], F32R, tag='pbig',
                                       name=f'pt_{b}_{hf}')
                    for kk in range(8):
                        nc.tensor.transpose(
                            pt[:, 128 * kk:128 * (kk + 1)],
                            xc[hf][:, kk * 128:(kk + 1) * 128],
                            ident[:],
                        )
                    nc.scalar.copy(out=xcT[hf][:], in_=pt[:])
                xcTs[b] = xcT

            h2s = {}

            def stage_b1(b):
                """mm1 + epilogue for batch b."""
                xcT = xcTs.pop(b)
                ph = []
                c0 = 0
                for cn in MM1_CHUNKS:
                    pht = phpool.tile([128, cn], FP32, tag='ph',
                                      name=f'ph_{b}_{c0}')
                    for k in range(FT):
                        nc.tensor.matmul(
                            pht[:],
                            xcT[k // 8][:, (k % 8) * 128:(k % 8 + 1) * 128],
                            w1ps[k][:, c0:c0 + cn],
                            start=(k == 0),
                            stop=(k == FT - 1),
                        )
                    ph.append((pht, c0, cn))
                    c0 += cn

                h2 = h2pool.tile([128, HID], F32R, tag='h2', name=f'h2_{b}')
                for pht, c0, cn in ph:
                    nc.vector.tensor_add(
                        h2[:, c0:c0 + cn], pht[:], b1rep[:, c0:c0 + cn]
                    )
                nc.scalar.activation(
                    h2[:], h2[:], mybir.ActivationFunctionType.Lrelu,
                    alpha=0.01,
                )
                eng = nc.vector if b % 2 == 0 else nc.gpsimd
                eng.tensor_mul(h2[:], h2[:], w2rep[:])
                h2s[b] = h2

            def stage_b2(b):
                """stage-2 pool-transpose + (every 4th) mm2 + store."""
                h2 = h2s.pop(b)
                pt2 = p2pool.tile([128, JT * T], FP32, tag='p2',
                                  name=f'pt2_{b}')
                for m in range(JT):
                    rows = 128 if m < JT - 1 else JLAST
                    nc.tensor.matmul(
                        pt2[0:rows, m * T:(m + 1) * T],
                        h2[:, m * 128:m * 128 + rows],
                        gpad[:, 96:128],
                        start=True, stop=True,
                    )
                g, bg = divmod(b, 4)
                if bg == 0:
                    hcT[g] = hcTpool.tile(
                        [128, JT * 128], F32R, tag='hcT', name=f'hcT{g}'
                    )
                dst = hcT[g].rearrange('p (m c) -> p m c', m=JT)[
                    :, :, 32 * bg:32 * (bg + 1)
                ]
                nc.scalar.copy(
                    out=dst, in_=pt2[:].rearrange('p (m c) -> p m c', m=JT)
                )

                if bg == 3:
                    po = p2pool.tile([128, NCLASS], FP32, tag='p2',
                                     name=f'po_{g}')
                    for m in range(JT):
                        rows = 128 if m < JT - 1 else JLAST
                        nc.tensor.matmul(
                            po[:],
                            hcT[g][0:rows, m * 128:(m + 1) * 128],
                            w2p[0:rows, m * NCLASS:(m + 1) * NCLASS],
                            start=(m == 0),
                            stop=(m == JT - 1),
                        )
                    ob = opool.tile([128, NCLASS], FP32, tag='ob')
                    nc.vector.tensor_add(ob[:], po[:], b2rep[:])
                    nc.scalar.activation(
                        ob[:], ob[:],
                        mybir.ActivationFunctionType.Lrelu, alpha=0.01,
                    )
                    nc.sync.dma_start(
                        out=y_flat[128 * g:128 * (g + 1), :], in_=ob[:]
                    )

            # Software pipeline: stage_b(b-1) is emitted between batches'
            # stage_a so the PE alternates mm1 with pool/transpose work
            # and never sits idle waiting on the scale-muls.  The conv1
            # scale for batch b+1 is emitted at iteration b so the DVE has
            # a full iteration of lead time; x(b+2) is prefetched so the
            # DMA queue stays ahead of the consumers.
            scale_c(cq_pre[0])
            scale_c(cq_pre[1])
            cqs = dict(cq_pre)
            for b in range(BPC):
                cq = cqs.pop(b)
                if b + 2 < BPC and (b + 2) not in cqs:
                    cqs[b + 2] = load_c(b + 2)
                if b == 2:
                    load_w2()
                if b >= 1:
                    stage_b1(b - 1)
                stage_a1(b, cq)
                if b >= 1:
                    stage_b2(b - 1)
                stage_a2(b)
                if b + 2 < BPC:
                    scale_c(cqs[b + 2])
            stage_b1(BPC - 1)
            stage_b2(BPC - 1)
    return nc


def _host_consts(conv1_w, pool1_w, pool1_b, prop1_W, prop1_B,
                 conv2_w, pool2_w, pool2_b, prop2_W, prop2_B):
    f32 = lambda a: np.ascontiguousarray(np.asarray(a, dtype=np.float32))
    gpad = np.zeros((128, 224), dtype=np.float32)
    gpad[np.arange(128), 96 + np.arange(128) // 4] = 1.0
    return {
        'w1rep': f32(np.tile(np.asarray(conv1_w), (32, 1))),
        'gpad': gpad,
        'ident': np.eye(128, dtype=np.float32),
        'w1p': f32(prop1_W),
        'b1rep': f32(np.broadcast_to(np.asarray(prop1_B), (128, HID))),
        'w2rep': f32(np.tile(np.asarray(conv2_w), (32, 1))),
        'w2p': f32(prop2_W),
        'b2rep': f32(np.broadcast_to(np.asarray(prop2_B), (128, NCLASS))),
    }


_COMPILED = {}


def run_on_cores(inputs, trace=False, **run_kwargs):
    x = np.ascontiguousarray(np.asarray(inputs['x'], dtype=np.float32))
    consts = _host_consts(**{k: v for k, v in inputs.items()
                             if k not in ('x', 'pooling_size')})
    if 'nc' not in _COMPILED:
        _COMPILED['nc'] = build_nc()
    nc = _COMPILED['nc']
    in_maps = []
    for c in range(N_CORES):
        m = {'xs': np.ascontiguousarray(x[c * BPC:(c + 1) * BPC])}
        m.update(consts)
        in_maps.append(m)
    res = run_bass_kernel_spmd(
        nc, in_maps, core_ids=list(range(N_CORES)), trace=trace, **run_kwargs
    )
    out = np.concatenate([res.results[c]['y'] for c in range(N_CORES)], axis=0)
    return out, res


def kernel(**inputs):
    out, _ = run_on_cores(inputs)
    return out



# revision 12
# speedup vs baseline: 1.0051x; 1.0051x over previous
"""Trainium2 Bass kernel for nn_DCGN_78967268704510.

Math: the reference's get_adjacent() builds a diagonal matrix (the faithful
buggy triple loop zeroes every off-diagonal), adds I, then symmetric-
normalizes; for a diagonal matrix D^-1/2 A D^-1/2 == I exactly (to fp32
ulps).  attn_pool feeds only get_adjacent, so the whole network collapses
to two fused stages:

  h   = leaky( (sum_p x[:,4s+p,:] * conv1_w[p,:]) @ prop1_W + prop1_B )
  out = leaky( (sum_p h[:,4t+p,:] * conv2_w[p,:]) @ prop2_W + prop2_B )

Sharding: pure data parallel, batch 64 -> 8 cores x 8 batches each.

v2.1 design (PE-throughput + DMA-overlap oriented):
  - single SP DMA ring with hand-interleaved order: consts/w1rep/b1rep,
    x(0), x(1), w1p[0:8], x(2), w1p[8:16], x(3), stage-2 weights,
    x(4..7) -- every tensor arrives just before its consumer needs it,
    so the PE never starves and the HWDGE FIFO never head-of-line
    blocks.
  - x arrives as one 2MB DMA per (batch, f-half) in a [128, q*4096+f]
    gathered layout: one SBUF tile per half-batch, 4-deep pool.
  - ~28 warmup matmuls keep the PE busy from t~0 so the HAM clock gate
    opens early and never re-throttles (cold PE halves matmul rate).
  - pool selectors stored contiguously per q (strided gpad windows cost
    ~300ns LDWEIGHTS vs ~190ns contiguous).
  - conv1 scale split 5:3 across DVE and GpSimd (measured rates ~1.4 vs
    ~2.8 us per [128,1024] under DMA load).
  - software pipeline: mm1(b-1) / pool(b) / stage2(b-1) / transpose(b)
    interleaved on the PE; scale(b+2) emitted a full iteration ahead.

All replicated weight/bias tiles are host-built exact fp32 (replicating
on-chip through fp32r matmuls costs ~5e-4 relative rounding and blows
the 2e-2 gate: measured rel err 1.93e-2 -> 2.21e-2).
"""
import sys

if '/opt/trn_rl_repo' not in sys.path:
    sys.path.insert(0, '/opt/trn_rl_repo')

import numpy as np

import concourse.bass as bass
import concourse.mybir as mybir
import concourse.tile as tile
from concourse.bass_utils import run_bass_kernel_spmd
from concourse.vector_clock import ScopedClock

N_CORES = 8
B, N, F, HID, NCLASS, P = 64, 512, 2048, 1100, 512, 4
BPC = B // N_CORES          # 8 batches per core
S = N // P                  # 128 stage-1 nodes
T = S // P                  # 32 stage-2 nodes
FT = F // 128               # 16 f-tiles
JT = (HID + 127) // 128     # 9 j-tiles, last has 76 rows
JLAST = HID - 128 * (JT - 1)
MM1_CHUNKS = (384, 384, 332)   # all >=256 so float32r runs at 1 cyc/row

FP32 = mybir.dt.float32
F32R = mybir.dt.float32r


class PatchedTileContext(tile.TileContext):
    """This container's walrus refuses ANY instruction carrying >1 sync
    wait (the TPB EVENTS struct has a single wait slot and the codegen
    won't split).  Split every multi-wait instruction into single-wait
    same-engine nops followed by the instruction with its last wait."""

    def _split_waits(self, inst):
        si = inst.sync_info
        waits = list(si.on_wait) if si and si.on_wait else []
        if len(waits) <= 1:
            return
        for w in waits[:-1]:
            nop = mybir.InstNoOp(
                name=self.nc.get_next_instruction_name(), ins=[], outs=[]
            )
            nop.engine = inst.engine
            nop.sync_info = mybir.SyncInfo(on_wait=[w], on_update=[])
            nop.bass_nofuse = True
            self._add_instruction(nop)
        inst.sync_info = mybir.SyncInfo(
            on_wait=[waits[-1]], on_update=list(si.on_update or [])
        )

    def _commit_instruction(self, inst, lazy_reg_writes=True):
        if inst.engine != mybir.EngineType.Unassigned:
            self._split_waits(inst)
        return super()._commit_instruction(inst, lazy_reg_writes)

    def _drain_and_barrier(self, tick_clock, wait_clock):
        probe = self.nc.sync.nop()
        wait_clock.add_sem_waits(
            probe.ins, ScopedClock({None: tick_clock.global_clock})
        )
        si = probe.ins.sync_info
        waits = list(si.on_wait) if si and si.on_wait else []
        if si and waits:
            probe.ins.sync_info = mybir.SyncInfo(
                on_wait=waits[:1], on_update=list(si.on_update or [])
            )
        for w in waits[1:]:
            n2 = self.nc.sync.nop()
            n2.ins.sync_info = mybir.SyncInfo(on_wait=[w], on_update=[])
        self.nc.sync.drain()
        self.nc.all_engine_barrier()
        assert self.sems is not None
        popped = self.nc._tile_sem_poison_stack.pop()
        assert popped is self._sem_poison
        self.nc.clear_and_free_semaphores(list(self.sems.allocated().values()))
        self.nc.all_engine_barrier()


def build_nc():
    nc = bass.Bass()
    xs_d = nc.dram_tensor('xs', [BPC, N, F], F32R, kind='ExternalInput')
    gq_d = nc.dram_tensor('gq', [128, 512], F32R, kind='ExternalInput')
    g2_d = nc.dram_tensor('g2', [128, 32], F32R, kind='ExternalInput')
    id_d = nc.dram_tensor('ident', [128, 128], F32R, kind='ExternalInput')
    w1rep_d = nc.dram_tensor('w1rep', [128, F], FP32, kind='ExternalInput')
    b1rep_d = nc.dram_tensor('b1rep', [128, HID], FP32, kind='ExternalInput')
    w2rep_d = nc.dram_tensor('w2rep', [128, HID], FP32, kind='ExternalInput')
    b2rep_d = nc.dram_tensor('b2rep', [128, NCLASS], FP32,
                             kind='ExternalInput')
    w1p_d = nc.dram_tensor('w1p', [F, HID], F32R, kind='ExternalInput')
    w2p_d = nc.dram_tensor('w2p', [HID, NCLASS], F32R, kind='ExternalInput')
    y_d = nc.dram_tensor('y', [BPC, T, NCLASS], FP32, kind='ExternalOutput')
    y_flat = y_d.rearrange('b t c -> (b t) c')   # [256, 512]

    with PatchedTileContext(nc) as tc:
        with (
            tc.tile_pool(name='wpool', bufs=1) as wpool,
            tc.tile_pool(name='cpool', bufs=4) as cpool,
            tc.tile_pool(name='xcpool', bufs=2) as xcpool,
            tc.tile_pool(name='xcTpool', bufs=3) as xcTpool,
            tc.tile_pool(name='h2pool', bufs=1) as h2pool,
            tc.tile_pool(name='hcTpool', bufs=1) as hcTpool,
            tc.tile_pool(name='opool', bufs=1) as opool,
            tc.tile_pool(name='pbig', bufs=2, space='PSUM') as pbigpool,
            tc.tile_pool(name='ph', bufs=3, space='PSUM') as phpool,
            tc.tile_pool(name='p2', bufs=1, space='PSUM') as p2pool,
        ):
            # ---- DMA order group 1: tiny consts + stage-1 scale/bias ----
            gq = wpool.tile([128, 512], F32R, tag='gq')
            nc.sync.dma_start(out=gq[:], in_=gq_d[:])
            g2 = wpool.tile([128, 32], F32R, tag='g2')
            nc.sync.dma_start(out=g2[:], in_=g2_d[:])
            ident = wpool.tile([128, 128], F32R, tag='ident')
            nc.sync.dma_start(out=ident[:], in_=id_d[:])
            w1rep = wpool.tile([128, F], FP32, tag='w1rep')
            nc.sync.dma_start(out=w1rep[:], in_=w1rep_d[:])
            b1rep = wpool.tile([128, HID], FP32, tag='b1rep')
            nc.sync.dma_start(out=b1rep[:], in_=b1rep_d[:])

            # ---- x loads: one 2MB DMA per (batch, f-half) ----
            cs = {}

            def load_c(b):
                src = xs_d[b].rearrange('(q p) (h f) -> p h q f', p=128, h=2)
                for h in range(2):
                    c = cpool.tile([128, 4096], F32R, tag='c',
                                   name=f'c_{b}_{h}')
                    nc.sync.dma_start(
                        out=c.rearrange('p (q f) -> p q f', q=4),
                        in_=src[:, h],
                    )
                    cs[(b, h)] = c

            load_c(0)
            load_c(1)

            # ---- DMA order group 2: first half of w1p ----
            w1ps = []
            for k in range(FT):
                w1ps.append(wpool.tile([128, HID], F32R, tag=f'w1p{k}',
                                       name=f'w1p{k}'))

            def load_w1p(k0, k1):
                for k in range(k0, k1):
                    nc.sync.dma_start(
                        out=w1ps[k][:], in_=w1p_d[k * 128:(k + 1) * 128, :]
                    )

            load_w1p(0, 8)

            # stage-2 weights (loaded at iteration 1)
            w2rep = wpool.tile([128, HID], FP32, tag='w2rep')
            b2rep = wpool.tile([128, NCLASS], FP32, tag='b2rep')
            w2p = wpool.tile([128, JT * NCLASS], F32R, tag='w2p')

            def load_w2():
                nc.sync.dma_start(out=w2rep[:], in_=w2rep_d[:])
                for m in range(JT):
                    rows = 128 if m < JT - 1 else JLAST
                    nc.sync.dma_start(
                        out=w2p[0:rows, m * NCLASS:(m + 1) * NCLASS],
                        in_=w2p_d[m * 128:m * 128 + rows, :],
                    )
                nc.sync.dma_start(out=b2rep[:], in_=b2rep_d[:])

            # ---- PE warmup: dummy matmuls during the initial DMA wait so
            #      the HAM clock gate reaches K=8/8 before batch 0 ----
            for w in range(4):
                warm_ps = p2pool.tile([128, 512], FP32, tag='p2',
                                      name=f'warm{w}')
                for i in range(7):
                    nc.tensor.matmul(warm_ps[:], ident[:], gq[:],
                                     start=(i == 0), stop=(i == 6))

            # ---- per-batch stages ----
            # scale: 8 units per batch = (half, q); DVE takes 5, GpSimd 3
            DVE_UNITS = {(0, 0), (0, 1), (0, 2), (1, 0), (1, 1)}

            def scale_c(b):
                for h in range(2):
                    cv = cs[(b, h)].rearrange('p (q f) -> p q f', q=4)
                    wsl = w1rep[:, h * 1024:(h + 1) * 1024]
                    for q in range(4):
                        eng = nc.vector if (h, q) in DVE_UNITS else nc.gpsimd
                        eng.tensor_mul(cv[:, q], cv[:, q], wsl)

            xcs = {}
            xcTs = {}
            h2s = {}
            hcT = [None, None]

            def pool_half(b, h):
                """pool f-half h of batch b -> xc[b][h] (SBUF [128,1024])."""
                cv = cs[(b, h)].rearrange('p (q f) -> p q f', q=4)
                pp = pbigpool.tile([128, 1024], FP32, tag='pbig',
                                   name=f'pp_{b}_{h}')
                for c2 in range(2):
                    for q in range(4):
                        nc.tensor.matmul(
                            pp[:, 512 * c2:512 * (c2 + 1)],
                            gq[:, 128 * q:128 * (q + 1)],
                            cv[:, q, 512 * c2:512 * (c2 + 1)],
                            start=(q == 0), stop=(q == 3),
                        )
                xc = xcpool.tile([128, 1024], F32R, tag='xc',
                                 name=f'xc_{b}_{h}')
                nc.scalar.copy(out=xc[:], in_=pp[:])
                xcs[(b, h)] = xc

            def transpose_half(b, h):
                xc = xcs.pop((b, h))
                pt = pbigpool.tile([128, 1024], F32R, tag='pbig',
                                   name=f'pt_{b}_{h}')
                for kk in range(8):
                    nc.tensor.transpose(
                        pt[:, 128 * kk:128 * (kk + 1)],
                        xc[:, kk * 128:(kk + 1) * 128],
                        ident[:],
                    )
                xcT = xcTpool.tile([128, 1024], F32R, tag='xcT',
                                   name=f'xcT_{b}_{h}')
                nc.scalar.copy(out=xcT[:], in_=pt[:])
                xcTs[(b, h)] = xcT

            def mm1(b):
                """mm1 + bias + leaky + conv2-scale for batch b."""
                xcT0 = xcTs.pop((b, 0))
                xcT1 = xcTs.pop((b, 1))
                xcT = (xcT0, xcT1)
                h2 = h2pool.tile([128, HID], F32R, tag='h2', name=f'h2_{b}')
                c0 = 0
                for cn in MM1_CHUNKS:
                    pht = phpool.tile([128, cn], FP32, tag='ph',
                                      name=f'ph_{b}_{c0}')
                    for k in range(FT):
                        nc.tensor.matmul(
                            pht[:],
                            xcT[k // 8][:, (k % 8) * 128:(k % 8 + 1) * 128],
                            w1ps[k][:, c0:c0 + cn],
                            start=(k == 0), stop=(k == FT - 1),
                        )
                    nc.vector.tensor_add(
                        h2[:, c0:c0 + cn], pht[:], b1rep[:, c0:c0 + cn]
                    )
                    nc.scalar.activation(
                        h2[:, c0:c0 + cn], h2[:, c0:c0 + cn],
                        mybir.ActivationFunctionType.Lrelu, alpha=0.01,
                    )
                    c0 += cn
                eng = nc.vector if b % 2 == 0 else nc.gpsimd
                eng.tensor_mul(h2[:], h2[:], w2rep[:])
                h2s[b] = h2

            def stage2(b):
                """stage-2 pool-transpose + (every 4th) mm2 + store."""
                h2 = h2s.pop(b)
                pt2 = p2pool.tile([128, JT * T], FP32, tag='p2',
                                  name=f'pt2_{b}')
                for m in range(JT):
                    rows = 128 if m < JT - 1 else JLAST
                    nc.tensor.matmul(
                        pt2[0:rows, m * T:(m + 1) * T],
                        h2[:, m * 128:m * 128 + rows],
                        g2[:],
                        start=True, stop=True,
                    )
                g, bg = divmod(b, 4)
                if bg == 0:
                    hcT[g] = hcTpool.tile(
                        [128, JT * 128], F32R, tag='hcT', name=f'hcT{g}'
                    )
                dst = hcT[g].rearrange('p (m c) -> p m c', m=JT)[
                    :, :, 32 * bg:32 * (bg + 1)
                ]
                src = pt2[:].rearrange('p (m c) -> p m c', m=JT)
                # region-exact: rows [JLAST:128] of the last j-block are
                # never written by the pt2 matmuls
                nc.scalar.copy(out=dst[:, 0:JT - 1], in_=src[:, 0:JT - 1])
                nc.scalar.copy(out=dst[0:JLAST, JT - 1:JT],
                               in_=src[0:JLAST, JT - 1:JT])

                if bg == 3:
                    po = p2pool.tile([128, NCLASS], FP32, tag='p2',
                                     name=f'po_{g}')
                    for m in range(JT):
                        rows = 128 if m < JT - 1 else JLAST
                        nc.tensor.matmul(
                            po[:],
                            hcT[g][0:rows, m * 128:(m + 1) * 128],
                            w2p[0:rows, m * NCLASS:(m + 1) * NCLASS],
                            start=(m == 0), stop=(m == JT - 1),
                        )
                    ob = opool.tile([128, NCLASS], FP32, tag='ob',
                                    name=f'ob_{g}')
                    nc.vector.tensor_add(ob[:], po[:], b2rep[:])
                    nc.scalar.activation(
                        ob[:], ob[:],
                        mybir.ActivationFunctionType.Lrelu, alpha=0.01,
                    )
                    nc.sync.dma_start(
                        out=y_flat[128 * g:128 * (g + 1), :], in_=ob[:]
                    )

            # ---- software pipeline ----
            scale_c(0)
            scale_c(1)
            for b in range(BPC):
                if b + 2 < BPC:
                    load_c(b + 2)
                if b == 0:
                    load_w1p(8, FT)
                if b == 1:
                    load_w2()
                if b >= 1:
                    mm1(b - 1)
                pool_half(b, 0)
                pool_half(b, 1)
                if b >= 1:
                    stage2(b - 1)
                transpose_half(b, 0)
                transpose_half(b, 1)
                if b + 2 < BPC:
                    scale_c(b + 2)
                cs.pop((b, 0))
                cs.pop((b, 1))
            mm1(BPC - 1)
            stage2(BPC - 1)
    return nc


def _host_consts(conv1_w, pool1_w, pool1_b, prop1_W, prop1_B,
                 conv2_w, pool2_w, pool2_b, prop2_W, prop2_B):
    f32 = lambda a: np.ascontiguousarray(np.asarray(a, dtype=np.float32))
    gq = np.zeros((128, 512), dtype=np.float32)
    n = np.arange(128)
    for q in range(4):
        gq[n, 128 * q + 32 * q + n // 4] = 1.0
    g2 = np.zeros((128, 32), dtype=np.float32)
    g2[n, n // 4] = 1.0
    return {
        'gq': gq,
        'g2': g2,
        'ident': np.eye(128, dtype=np.float32),
        'w1rep': f32(np.tile(np.asarray(conv1_w), (32, 1))),
        'b1rep': f32(np.broadcast_to(np.asarray(prop1_B), (128, HID))),
        'w2rep': f32(np.tile(np.asarray(conv2_w), (32, 1))),
        'b2rep': f32(np.broadcast_to(np.asarray(prop2_B), (128, NCLASS))),
        'w1p': f32(prop1_W),
        'w2p': f32(prop2_W),
    }


_COMPILED = {}


def run_on_cores(inputs, trace=False, **run_kwargs):
    x = np.ascontiguousarray(np.asarray(inputs['x'], dtype=np.float32))
    consts = _host_consts(**{k: v for k, v in inputs.items()
                             if k not in ('x', 'pooling_size')})
    if 'nc' not in _COMPILED:
        _COMPILED['nc'] = build_nc()
    nc = _COMPILED['nc']
    in_maps = []
    for c in range(N_CORES):
        m = {'xs': np.ascontiguousarray(x[c * BPC:(c + 1) * BPC])}
        m.update(consts)
        in_maps.append(m)
    res = run_bass_kernel_spmd(
        nc, in_maps, core_ids=list(range(N_CORES)), trace=trace, **run_kwargs
    )
    out = np.concatenate([res.results[c]['y'] for c in range(N_CORES)], axis=0)
    return out, res


def kernel(**inputs):
    out, _ = run_on_cores(inputs)
    return out


# revision 16
# speedup vs baseline: 1.1310x; 1.1253x over previous
"""Trainium2 Bass kernel for nn_DCGN_78967268704510.

Math: the reference's get_adjacent() builds a diagonal matrix (the faithful
buggy triple loop zeroes every off-diagonal), adds I, then symmetric-
normalizes; for a diagonal matrix D^-1/2 A D^-1/2 == I exactly (to fp32
ulps).  attn_pool feeds only get_adjacent, so the whole network collapses
to two fused stages:

  h   = leaky( (sum_p x[:,4s+p,:] * conv1_w[p,:]) @ prop1_W + prop1_B )
  out = leaky( (sum_p h[:,4t+p,:] * conv2_w[p,:]) @ prop2_W + prop2_B )

Sharding: pure data parallel, batch 64 -> 8 cores x 8 batches each.

v2.1 design (PE-throughput + DMA-overlap oriented):
  - single SP DMA ring with hand-interleaved order: consts/w1rep/b1rep,
    x(0), x(1), w1p[0:8], x(2), w1p[8:16], x(3), stage-2 weights,
    x(4..7) -- every tensor arrives just before its consumer needs it,
    so the PE never starves and the HWDGE FIFO never head-of-line
    blocks.
  - x arrives as one 2MB DMA per (batch, f-half) in a [128, q*4096+f]
    gathered layout: one SBUF tile per half-batch, 4-deep pool.
  - ~28 warmup matmuls keep the PE busy from t~0 so the HAM clock gate
    opens early and never re-throttles (cold PE halves matmul rate).
  - pool selectors stored contiguously per q (strided gpad windows cost
    ~300ns LDWEIGHTS vs ~190ns contiguous).
  - conv1 scale split 5:3 across DVE and GpSimd (measured rates ~1.4 vs
    ~2.8 us per [128,1024] under DMA load).
  - software pipeline: mm1(b-1) / pool(b) / stage2(b-1) / transpose(b)
    interleaved on the PE; scale(b+2) emitted a full iteration ahead.

All replicated weight/bias tiles are host-built exact fp32 (replicating
on-chip through fp32r matmuls costs ~5e-4 relative rounding and blows
the 2e-2 gate: measured rel err 1.93e-2 -> 2.21e-2).
"""
import sys

if '/opt/trn_rl_repo' not in sys.path:
    sys.path.insert(0, '/opt/trn_rl_repo')

import numpy as np

import concourse.bass as bass
import concourse.mybir as mybir
import concourse.tile as tile
from concourse.bass_utils import run_bass_kernel_spmd
from concourse.vector_clock import ScopedClock

N_CORES = 8
B, N, F, HID, NCLASS, P = 64, 512, 2048, 1100, 512, 4
BPC = B // N_CORES          # 8 batches per core
S = N // P                  # 128 stage-1 nodes
T = S // P                  # 32 stage-2 nodes
FT = F // 128               # 16 f-tiles
JT = (HID + 127) // 128     # 9 j-tiles, last has 76 rows
JLAST = HID - 128 * (JT - 1)
MM1_CHUNKS = (384, 384, 332)   # all >=256 so float32r runs at 1 cyc/row

FP32 = mybir.dt.float32
F32R = mybir.dt.float32r


class PatchedTileContext(tile.TileContext):
    """This container's walrus refuses ANY instruction carrying >1 sync
    wait (the TPB EVENTS struct has a single wait slot and the codegen
    won't split).  Split every multi-wait instruction into single-wait
    same-engine nops followed by the instruction with its last wait."""

    def _split_waits(self, inst):
        si = inst.sync_info
        waits = list(si.on_wait) if si and si.on_wait else []
        if len(waits) <= 1:
            return
        for w in waits[:-1]:
            nop = mybir.InstNoOp(
                name=self.nc.get_next_instruction_name(), ins=[], outs=[]
            )
            nop.engine = inst.engine
            nop.sync_info = mybir.SyncInfo(on_wait=[w], on_update=[])
            nop.bass_nofuse = True
            self._add_instruction(nop)
        inst.sync_info = mybir.SyncInfo(
            on_wait=[waits[-1]], on_update=list(si.on_update or [])
        )

    def _commit_instruction(self, inst, lazy_reg_writes=True):
        if inst.engine != mybir.EngineType.Unassigned:
            self._split_waits(inst)
        return super()._commit_instruction(inst, lazy_reg_writes)

    def _drain_and_barrier(self, tick_clock, wait_clock):
        probe = self.nc.sync.nop()
        wait_clock.add_sem_waits(
            probe.ins, ScopedClock({None: tick_clock.global_clock})
        )
        si = probe.ins.sync_info
        waits = list(si.on_wait) if si and si.on_wait else []
        if si and waits:
            probe.ins.sync_info = mybir.SyncInfo(
                on_wait=waits[:1], on_update=list(si.on_update or [])
            )
        for w in waits[1:]:
            n2 = self.nc.sync.nop()
            n2.ins.sync_info = mybir.SyncInfo(on_wait=[w], on_update=[])
        self.nc.sync.drain()
        self.nc.all_engine_barrier()
        assert self.sems is not None
        popped = self.nc._tile_sem_poison_stack.pop()
        assert popped is self._sem_poison
        self.nc.clear_and_free_semaphores(list(self.sems.allocated().values()))
        self.nc.all_engine_barrier()


def build_nc():
    nc = bass.Bass()
    xs_d = nc.dram_tensor('xs', [BPC, N, F], F32R, kind='ExternalInput')
    gq_d = nc.dram_tensor('gq', [128, 512], F32R, kind='ExternalInput')
    g2_d = nc.dram_tensor('g2', [128, 32], F32R, kind='ExternalInput')
    id_d = nc.dram_tensor('ident', [128, 128], F32R, kind='ExternalInput')
    w1rep_d = nc.dram_tensor('w1rep', [128, F], FP32, kind='ExternalInput')
    b1rep_d = nc.dram_tensor('b1rep', [128, HID], FP32, kind='ExternalInput')
    w2rep_d = nc.dram_tensor('w2rep', [128, HID], FP32, kind='ExternalInput')
    b2rep_d = nc.dram_tensor('b2rep', [128, NCLASS], FP32,
                             kind='ExternalInput')
    w1p_d = nc.dram_tensor('w1p', [F, HID], F32R, kind='ExternalInput')
    w2p_d = nc.dram_tensor('w2p', [HID, NCLASS], F32R, kind='ExternalInput')
    y_d = nc.dram_tensor('y', [BPC, T, NCLASS], FP32, kind='ExternalOutput')
    y_flat = y_d.rearrange('b t c -> (b t) c')   # [256, 512]

    with PatchedTileContext(nc) as tc:
        with (
            tc.tile_pool(name='wpool', bufs=1) as wpool,
            tc.tile_pool(name='cpool', bufs=4) as cpool,
            tc.tile_pool(name='xcpool', bufs=2) as xcpool,
            tc.tile_pool(name='xcTpool', bufs=3) as xcTpool,
            tc.tile_pool(name='h2pool', bufs=1) as h2pool,
            tc.tile_pool(name='hcTpool', bufs=1) as hcTpool,
            tc.tile_pool(name='opool', bufs=1) as opool,
            tc.tile_pool(name='pbig', bufs=2, space='PSUM') as pbigpool,
            tc.tile_pool(name='ph', bufs=3, space='PSUM') as phpool,
            tc.tile_pool(name='p2', bufs=1, space='PSUM') as p2pool,
        ):
            # ---- DMA order group 1: tiny consts + stage-1 scale/bias ----
            gq = wpool.tile([128, 512], F32R, tag='gq')
            nc.sync.dma_start(out=gq[:], in_=gq_d[:])
            g2 = wpool.tile([128, 32], F32R, tag='g2')
            nc.sync.dma_start(out=g2[:], in_=g2_d[:])
            ident = wpool.tile([128, 128], F32R, tag='ident')
            nc.sync.dma_start(out=ident[:], in_=id_d[:])
            w1rep = wpool.tile([128, F], FP32, tag='w1rep')
            nc.sync.dma_start(out=w1rep[:], in_=w1rep_d[:])
            b1rep = wpool.tile([128, HID], FP32, tag='b1rep')
            nc.sync.dma_start(out=b1rep[:], in_=b1rep_d[:])

            # ---- x loads: one 2MB DMA per (batch, f-half) ----
            cs = {}

            def load_ch(b, h):
                src = xs_d[b].rearrange('(q p) (h f) -> p h q f', p=128, h=2)
                c = cpool.tile([128, 4096], F32R, tag='c',
                               name=f'c_{b}_{h}')
                nc.sync.dma_start(
                    out=c.rearrange('p (q f) -> p q f', q=4),
                    in_=src[:, h],
                )
                cs[(b, h)] = c

            def load_c(b):
                load_ch(b, 0)
                load_ch(b, 1)

            load_c(0)
            load_c(1)

            # ---- DMA order group 2: first chunk of w1p ----
            w1ps = []
            for k in range(FT):
                w1ps.append(wpool.tile([128, HID], F32R, tag=f'w1p{k}',
                                       name=f'w1p{k}'))

            def load_w1p(k0, k1):
                for k in range(k0, k1):
                    nc.sync.dma_start(
                        out=w1ps[k][:], in_=w1p_d[k * 128:(k + 1) * 128, :]
                    )

            load_w1p(0, 6)

            # stage-2 weights (loaded at iteration 1)
            w2rep = wpool.tile([128, HID], FP32, tag='w2rep')
            b2rep = wpool.tile([128, NCLASS], FP32, tag='b2rep')
            w2p = wpool.tile([128, JT * NCLASS], F32R, tag='w2p')

            def load_w2():
                nc.sync.dma_start(out=w2rep[:], in_=w2rep_d[:])
                for m in range(JT):
                    rows = 128 if m < JT - 1 else JLAST
                    nc.sync.dma_start(
                        out=w2p[0:rows, m * NCLASS:(m + 1) * NCLASS],
                        in_=w2p_d[m * 128:m * 128 + rows, :],
                    )
                nc.sync.dma_start(out=b2rep[:], in_=b2rep_d[:])

            # ---- PE warmup: dummy matmuls during the initial DMA wait so
            #      the HAM clock gate reaches K=8/8 before batch 0 ----
            for w in range(4):
                warm_ps = p2pool.tile([128, 512], FP32, tag='p2',
                                      name=f'warm{w}')
                for i in range(7):
                    nc.tensor.matmul(warm_ps[:], ident[:], gq[:],
                                     start=(i == 0), stop=(i == 6))

            # ---- per-batch stages ----
            # ALL elementwise work runs on DVE: any concurrent GpSimd
            # tensor op degrades DVE from ~1.2us to ~3.4us per [128,1024]
            # (measured), so GpSimd's 3.3us/op "help" is net-negative.

            def scale_c(b):
                for h in range(2):
                    cv = cs[(b, h)].rearrange('p (q f) -> p q f', q=4)
                    wsl = w1rep[:, h * 1024:(h + 1) * 1024]
                    for q in range(4):
                        nc.vector.tensor_mul(cv[:, q], cv[:, q], wsl)

            xcs = {}
            xcTs = {}
            h2s = {}
            hcT = [None, None]

            def pool_half(b, h):
                """pool f-half h of batch b -> xc[b][h] (SBUF [128,1024])."""
                cv = cs[(b, h)].rearrange('p (q f) -> p q f', q=4)
                pp = pbigpool.tile([128, 1024], FP32, tag='pbig',
                                   name=f'pp_{b}_{h}')
                for c2 in range(2):
                    for q in range(4):
                        nc.tensor.matmul(
                            pp[:, 512 * c2:512 * (c2 + 1)],
                            gq[:, 128 * q:128 * (q + 1)],
                            cv[:, q, 512 * c2:512 * (c2 + 1)],
                            start=(q == 0), stop=(q == 3),
                        )
                xc = xcpool.tile([128, 1024], F32R, tag='xc',
                                 name=f'xc_{b}_{h}')
                nc.scalar.copy(out=xc[:], in_=pp[:])
                xcs[(b, h)] = xc

            def transpose_half(b, h):
                xc = xcs.pop((b, h))
                pt = pbigpool.tile([128, 1024], F32R, tag='pbig',
                                   name=f'pt_{b}_{h}')
                for kk in range(8):
                    nc.tensor.transpose(
                        pt[:, 128 * kk:128 * (kk + 1)],
                        xc[:, kk * 128:(kk + 1) * 128],
                        ident[:],
                    )
                xcT = xcTpool.tile([128, 1024], F32R, tag='xcT',
                                   name=f'xcT_{b}_{h}')
                nc.scalar.copy(out=xcT[:], in_=pt[:])
                xcTs[(b, h)] = xcT

            def mm1(b):
                """mm1 + bias + leaky + conv2-scale for batch b."""
                xcT0 = xcTs.pop((b, 0))
                xcT1 = xcTs.pop((b, 1))
                xcT = (xcT0, xcT1)
                h2 = h2pool.tile([128, HID], F32R, tag='h2', name=f'h2_{b}')
                c0 = 0
                for cn in MM1_CHUNKS:
                    pht = phpool.tile([128, cn], FP32, tag='ph',
                                      name=f'ph_{b}_{c0}')
                    for k in range(FT):
                        nc.tensor.matmul(
                            pht[:],
                            xcT[k // 8][:, (k % 8) * 128:(k % 8 + 1) * 128],
                            w1ps[k][:, c0:c0 + cn],
                            start=(k == 0), stop=(k == FT - 1),
                        )
                    nc.vector.tensor_add(
                        h2[:, c0:c0 + cn], pht[:], b1rep[:, c0:c0 + cn]
                    )
                    nc.scalar.activation(
                        h2[:, c0:c0 + cn], h2[:, c0:c0 + cn],
                        mybir.ActivationFunctionType.Lrelu, alpha=0.01,
                    )
                    c0 += cn
                nc.vector.tensor_mul(h2[:], h2[:], w2rep[:])
                h2s[b] = h2

            def stage2(b):
                """stage-2 pool-transpose + (every 4th) mm2 + store."""
                h2 = h2s.pop(b)
                pt2 = p2pool.tile([128, JT * T], FP32, tag='p2',
                                  name=f'pt2_{b}')
                for m in range(JT):
                    rows = 128 if m < JT - 1 else JLAST
                    nc.tensor.matmul(
                        pt2[0:rows, m * T:(m + 1) * T],
                        h2[:, m * 128:m * 128 + rows],
                        g2[:],
                        start=True, stop=True,
                    )
                g, bg = divmod(b, 4)
                if bg == 0:
                    hcT[g] = hcTpool.tile(
                        [128, JT * 128], F32R, tag='hcT', name=f'hcT{g}'
                    )
                dst = hcT[g].rearrange('p (m c) -> p m c', m=JT)[
                    :, :, 32 * bg:32 * (bg + 1)
                ]
                src = pt2[:].rearrange('p (m c) -> p m c', m=JT)
                # region-exact: rows [JLAST:128] of the last j-block are
                # never written by the pt2 matmuls
                nc.scalar.copy(out=dst[:, 0:JT - 1], in_=src[:, 0:JT - 1])
                nc.scalar.copy(out=dst[0:JLAST, JT - 1:JT],
                               in_=src[0:JLAST, JT - 1:JT])

                if bg == 3:
                    po = p2pool.tile([128, NCLASS], FP32, tag='p2',
                                     name=f'po_{g}')
                    for m in range(JT):
                        rows = 128 if m < JT - 1 else JLAST
                        nc.tensor.matmul(
                            po[:],
                            hcT[g][0:rows, m * 128:(m + 1) * 128],
                            w2p[0:rows, m * NCLASS:(m + 1) * NCLASS],
                            start=(m == 0), stop=(m == JT - 1),
                        )
                    ob = opool.tile([128, NCLASS], FP32, tag='ob',
                                    name=f'ob_{g}')
                    nc.vector.tensor_add(ob[:], po[:], b2rep[:])
                    nc.scalar.activation(
                        ob[:], ob[:],
                        mybir.ActivationFunctionType.Lrelu, alpha=0.01,
                    )
                    nc.sync.dma_start(
                        out=y_flat[128 * g:128 * (g + 1), :], in_=ob[:]
                    )

            # ---- software pipeline ----
            scale_c(0)
            scale_c(1)
            for b in range(BPC):
                if b == 0:
                    # interleave the rest of w1p with x(2) at half-batch
                    # granularity so neither stream starves its consumer
                    load_ch(2, 0)
                    load_w1p(6, 11)
                    load_ch(2, 1)
                    load_w1p(11, FT)
                elif b + 2 < BPC:
                    load_c(b + 2)
                if b == 1:
                    load_w2()
                if b >= 1:
                    mm1(b - 1)
                pool_half(b, 0)
                pool_half(b, 1)
                if b >= 1:
                    stage2(b - 1)
                transpose_half(b, 0)
                transpose_half(b, 1)
                if b + 2 < BPC:
                    scale_c(b + 2)
                cs.pop((b, 0))
                cs.pop((b, 1))
            mm1(BPC - 1)
            stage2(BPC - 1)
    return nc


def _host_consts(conv1_w, pool1_w, pool1_b, prop1_W, prop1_B,
                 conv2_w, pool2_w, pool2_b, prop2_W, prop2_B):
    f32 = lambda a: np.ascontiguousarray(np.asarray(a, dtype=np.float32))
    gq = np.zeros((128, 512), dtype=np.float32)
    n = np.arange(128)
    for q in range(4):
        gq[n, 128 * q + 32 * q + n // 4] = 1.0
    g2 = np.zeros((128, 32), dtype=np.float32)
    g2[n, n // 4] = 1.0
    return {
        'gq': gq,
        'g2': g2,
        'ident': np.eye(128, dtype=np.float32),
        'w1rep': f32(np.tile(np.asarray(conv1_w), (32, 1))),
        'b1rep': f32(np.broadcast_to(np.asarray(prop1_B), (128, HID))),
        'w2rep': f32(np.tile(np.asarray(conv2_w), (32, 1))),
        'b2rep': f32(np.broadcast_to(np.asarray(prop2_B), (128, NCLASS))),
        'w1p': f32(prop1_W),
        'w2p': f32(prop2_W),
    }


_COMPILED = {}


def run_on_cores(inputs, trace=False, **run_kwargs):
    x = np.ascontiguousarray(np.asarray(inputs['x'], dtype=np.float32))
    consts = _host_consts(**{k: v for k, v in inputs.items()
                             if k not in ('x', 'pooling_size')})
    if 'nc' not in _COMPILED:
        _COMPILED['nc'] = build_nc()
    nc = _COMPILED['nc']
    in_maps = []
    for c in range(N_CORES):
        m = {'xs': np.ascontiguousarray(x[c * BPC:(c + 1) * BPC])}
        m.update(consts)
        in_maps.append(m)
    res = run_bass_kernel_spmd(
        nc, in_maps, core_ids=list(range(N_CORES)), trace=trace, **run_kwargs
    )
    out = np.concatenate([res.results[c]['y'] for c in range(N_CORES)], axis=0)
    return out, res


def kernel(**inputs):
    out, _ = run_on_cores(inputs)
    return out
